# revision 18
# baseline (speedup 1.0000x reference)
"""Masked multi-head attention (B=4, T=2048, D=1024, H=16) on 8 trn2 NeuronCores.

Sharding: core c handles batch b = c//2 and head-group g = c%2 (8 heads, 512
of the 1024 model dims).  Each core runs the fused QKV projection for its
head-group over its batch, causal+padding-masked attention for its 8 heads,
and a partial out-projection (its 512 rows of W_o).  The two cores of a batch
produce additive partials of y[b]; the host sums the pair (0.6% of FLOPs).

Device algorithm (per core), all matmuls bf16 with f32 PSUM accumulation:
  - qT,kT  = (x @ Wq|k)^T computed directly in [dims, tok] layout
             (lhsT = W chunk, rhs = xT chunk), bias added per-partition.
  - V      computed in natural [tok, dims] layout (lhsT = xT chunk,
             rhs = Wv), packed into V_aug = [V | 1] (even heads) or [1 | V]
             (odd heads) so A@V_aug also yields the softmax row-sums
             replicated across 64 partitions.
  - scores S^T[k, q] per 128-key block kb: lhsT = kT block, rhs = qT.
             Keys >= 1792 are fully padded -> those blocks never computed.
             Causal: only q >= 128*kb computed; exp(S/8) via ScalarE into
             bf16; diagonal band masked multiplicatively.
  - ctx^T  accumulated over key blocks in PSUM; row-sums come free via the
             V_aug ones-columns; reciprocal on VectorE; normalize into bf16.
  - y      = ctx @ W_o rows (natural layout) + b_o broadcast, f32 out.

Scheduling: everything is emitted as one interleaved stream.  Attention is
processed q-tile-major per head pair with scores -> exp -> A@V interleaved at
key-block granularity; QKV projection tiles, V tiles and the out-projection
are woven between key blocks as PE filler so the tensor engine never idles
long enough for the HAM clock gate to drop it to 1.2 GHz.  Host-side, wq/xT
are packed into the exact SBUF block layout so every DMA wave is one large
contiguous transfer ordered by first use.
"""

import os
import sys

sys.path.insert(0, "/opt/trn_rl_repo")

from contextlib import ExitStack

import ml_dtypes
import numpy as np

import concourse.bass as bass
import concourse.tile as tile
from concourse import bacc, mybir
from concourse.bass_utils import run_bass_kernel_spmd

B, T, D, H, HD = 4, 2048, 1024, 16, 64
N_CORES = 8
NH = H // 2            # heads per core = 8
GD = NH * HD           # head-group width = 512
TK = 14                # valid 128-key blocks (keys < 1792; rest padded)
NPAD = 256             # padded key positions at the end
BF16 = mybir.dt.bfloat16
F32 = mybir.dt.float32
AF = mybir.ActivationFunctionType

_CACHE = {}


def _build():
    nc = bacc.Bacc("TRN2", target_bir_lowering=False, debug=False,
                   num_devices=N_CORES)
    # xT packed as [128, (nt, d) blocks of 512]; wq packed as
    # [128, m0|m4|V|m1|m5|m2|m6|m3|m7 blocks] -- both host-reordered so every
    # DMA wave is fully contiguous (large descriptors, ordered by first use).
    xT_d = nc.dram_tensor("xT", [128, 8 * T], BF16, kind="ExternalInput").ap()
    wqkv_d = nc.dram_tensor("wqkv", [128, 8 * 3 * GD // 128 * 128], BF16,
                            kind="ExternalInput").ap()
    wo_d = nc.dram_tensor("wo", [GD, D], BF16, kind="ExternalInput").ap()
    bqk_d = nc.dram_tensor("bqk", [128, 8], F32, kind="ExternalInput").ap()
    bv_d = nc.dram_tensor("bv", [GD], F32, kind="ExternalInput").ap()
    bo_d = nc.dram_tensor("bo", [D], F32, kind="ExternalInput").ap()
    y_d = nc.dram_tensor("y", [T, D], F32, kind="ExternalOutput").ap()

    def bcast128(src_ap):
        """DMA access pattern replicating a 1-D dram vector over 128 partitions."""
        return bass.AP(tensor=src_ap.tensor, offset=src_ap.offset,
                       ap=[[0, 128]] + list(src_ap.ap))

    with tile.TileContext(nc) as tc, ExitStack() as ctx:
        pers = ctx.enter_context(tc.tile_pool(name="pers", bufs=1))
        ps_pool = ctx.enter_context(tc.tile_pool(name="ps", bufs=2, space="PSUM"))
        esp = ctx.enter_context(tc.tile_pool(name="es", bufs=6))
        stgp = ctx.enter_context(tc.tile_pool(name="stg", bufs=2))
        nrmp = ctx.enter_context(tc.tile_pool(name="nrm", bufs=1))
        yp = ctx.enter_context(tc.tile_pool(name="yp", bufs=2))

        # ---- persistent tiles ----
        wo_sb = pers.tile([128, 4, D], BF16)          # W_o rows, 4 chunks of 128
        bqk_sb = pers.tile([128, 8], F32)             # q|k bias per col-tile
        bv_bc = pers.tile([128, GD], F32)             # v bias bcast over tokens
        bo_bc = pers.tile([128, D], F32)              # out bias bcast over tokens
        band = pers.tile([128, 1024], BF16)           # band[k, i] = 1 iff i-512 >= k
        qk_sb = pers.tile([128, 8, T], BF16)          # m<4: qT pairs, m>=4: kT
        vaug = pers.tile([128, 2, 4, TK, 128], BF16)  # V_aug[par, hp, key chunk]
        xT_sb = pers.tile([128, 8 * T], BF16)         # packed (nt, d) blocks
        wq_sb = pers.tile([128, 12 * 1024], BF16)     # packed m/V blocks

        QKOFF = {0: 0, 4: 1024, 1: 6144, 5: 7168, 2: 8192, 6: 9216,
                 3: 10240, 7: 11264}
        VOFF = 2048

        def wq_qk(m, d8):
            return wq_sb[:, QKOFF[m] + 128 * d8:QKOFF[m] + 128 * (d8 + 1)]

        def xT_nt(nt, d8):
            return xT_sb[:, (nt * 8 + d8) * 512:(nt * 8 + d8) * 512 + 512]
        ctxn = pers.tile([128, 4, 4, 512], BF16)      # normalized ctx^T chunks

        # ---- loads: contiguous waves ordered by first use, split in half so
        #      two DMA queues work each wave ----
        def wave(sb, dram, lo, hi):
            mid = (lo + hi) // 2
            nc.sync.dma_start(out=sb[:, lo:mid], in_=dram[:, lo:mid])
            nc.sync.dma_start(out=sb[:, mid:hi], in_=dram[:, mid:hi])

        wave(wq_sb, wqkv_d, 0, 2048)            # m0 + m4
        wave(xT_sb, xT_d, 0, 2048)              # nt0 d0..3
        wave(xT_sb, xT_d, 2048, 4096)           # nt0 d4..7
        nc.sync.dma_start(out=bqk_sb[:], in_=bqk_d)
        nc.sync.dma_start(out=bv_bc[:], in_=bcast128(bv_d))
        wave(wq_sb, wqkv_d, 2048, 6144)         # V columns
        wave(xT_sb, xT_d, 4096, 8192)           # nt1
        wave(wq_sb, wqkv_d, 6144, 8192)         # m1 + m5
        wave(xT_sb, xT_d, 8192, 12288)          # nt2
        wave(wq_sb, wqkv_d, 8192, 10240)        # m2 + m6
        wave(xT_sb, xT_d, 12288, 16384)         # nt3
        wave(wq_sb, wqkv_d, 10240, 12288)       # m3 + m7
        for c4 in range(4):
            nc.sync.dma_start(out=wo_sb[:, c4, :], in_=wo_d[128 * c4:128 * (c4 + 1), :])
        nc.sync.dma_start(out=bo_bc[:], in_=bcast128(bo_d))
        nc.vector.memset(band[:], 1.0)
        # keep 1.0 where (i - 512) - k >= 0 else 0.0
        nc.gpsimd.affine_select(out=band[:], in_=band[:],
                                compare_op=mybir.AluOpType.is_ge, fill=0.0,
                                base=-512, pattern=[[1, 1024]], channel_multiplier=-1)
        nc.vector.memset(vaug[:, 0, :, :, 64:128], 1.0)   # even heads: [V | 1]
        nc.vector.memset(vaug[:, 1, :, :, 0:64], 1.0)     # odd heads:  [1 | V]

        # ---- QKV projection pieces, emitted as PE fillers ----
        def qk_tile(m, nt):
            # k columns (m >= 4) beyond token 1792 are fully padded: never read
            w = 256 if (m >= 4 and nt == 3) else 512
            ps = ps_pool.tile([128, 512], F32, tag="p1", name=f"p1_{m}_{nt}")
            for d8 in range(8):
                nc.tensor.matmul(ps[:, 0:w], lhsT=wq_qk(m, d8),
                                 rhs=xT_nt(nt, d8)[:, 0:w],
                                 start=(d8 == 0), stop=(d8 == 7))
            nc.vector.tensor_scalar_add(qk_sb[:, m, 512 * nt:512 * nt + w],
                                        ps[:, 0:w], bqk_sb[:, m:m + 1])

        def v_tile(t16):
            ps = ps_pool.tile([128, 512], F32, tag="p1", name=f"p1v_{t16}")
            nt, to = t16 // 4, 128 * (t16 % 4)
            for d8 in range(8):
                nc.tensor.matmul(ps[:],
                                 lhsT=xT_sb[:, (nt * 8 + d8) * 512 + to:(nt * 8 + d8) * 512 + to + 128],
                                 rhs=wq_sb[:, VOFF + 512 * d8:VOFF + 512 * (d8 + 1)],
                                 start=(d8 == 0), stop=(d8 == 7))
            psv = ps.rearrange("p (hp par d) -> p hp par d", par=2, d=64)
            bvv = bv_bc.rearrange("p (hp par d) -> p hp par d", par=2, d=64)
            nc.vector.tensor_add(vaug[:, 0, :, t16, 0:64], psv[:, :, 0, :],
                                 bvv[:, :, 0, :])
            nc.vector.tensor_add(vaug[:, 1, :, t16, 64:128], psv[:, :, 1, :],
                                 bvv[:, :, 1, :])

        stg_tiles = {}  # h -> stage tile

        def attention_qt(c, qt, fillers=()):
            """Scores + exp + A@V_aug for q-tile qt of head pair c, interleaved
            per key block so ScalarE exp overlaps the PE matmuls.  The two
            heads occupy PE row-groups 0/64 (concurrent matmuls) and the two
            halves of shared score/exp tiles.  `fillers` are independent PE
            work (QKV tiles / out-projection) woven between key blocks to
            absorb the exp latency."""
            kmax = min(4 * qt + 3, TK - 1)
            fillers = list(fillers)
            fill_every = max(1, (kmax + 1) // (len(fillers) + 1)) if fillers else 0
            cps = [ps_pool.tile([128, 512], F32, tag="cps", name=f"cps_{c}_{qt}_{i}")
                   for i in range(2)]
            for kb in range(kmax + 1):
                if c == 0 and qt == kb // 4:   # JIT V chunks during pair 0
                    v_tile(kb)
                # diagonal blocks only need columns q >= 128*kb of the q-tile
                off = max(0, 128 * kb - 512 * qt)
                psc = ps_pool.tile([128, 1024], F32, tag="sc", name=f"sc_{c}_{qt}_{kb}")
                for par in (0, 1):
                    r = 64 * par
                    nc.tensor.matmul(
                        psc[:, 512 * par + off:512 * (par + 1)],
                        lhsT=qk_sb[r:r + 64, 4 + c, 128 * kb:128 * (kb + 1)],
                        rhs=qk_sb[r:r + 64, c, 512 * qt + off:512 * (qt + 1)],
                        start=True, stop=True)
                est = esp.tile([128, 1024], BF16, tag="es", name=f"es_{c}_{qt}_{kb}")
                # full width even for diagonal blocks: the dead columns read
                # stale PSUM, but nothing downstream ever reads them
                nc.scalar.activation(est[:], psc[:], AF.Exp,
                                     scale=float(1.0 / np.sqrt(HD)))
                if kb >= 4 * qt:  # mask the causal triangle of the diagonal block
                    for par in (0, 1):
                        nc.vector.tensor_mul(est[:, 512 * par + off:512 * (par + 1)],
                                             est[:, 512 * par + off:512 * (par + 1)],
                                             band[:, 512:1024 - off])
                for par in (0, 1):
                    nc.tensor.matmul(cps[par][:, off:512],
                                     lhsT=vaug[:, par, c, kb, :],
                                     rhs=est[:, 512 * par + off:512 * (par + 1)],
                                     start=(kb == 0), stop=(kb == kmax))
                if fillers and fill_every and kb % fill_every == fill_every - 1:
                    fillers.pop(0)()
            for f in fillers:
                f()
            for par in (0, 1):
                h = 2 * c + par
                if qt == 0:
                    stg_tiles[h] = stgp.tile([128, 4, 512], F32, tag="stg",
                                             name=f"stg_{h}")
                nc.vector.tensor_copy(stg_tiles[h][:, qt, :], cps[par][:])

        def normalize(c, qt):
            he, ho = stg_tiles[2 * c], stg_tiles[2 * c + 1]
            sums = nrmp.tile([128, 512], F32, tag="sums", name=f"sums_{c}_{qt}",
                             bufs=2)
            # even head: ctx rows 0:64, sums rows 64:128 (V_aug = [V|1])
            # odd head:  sums rows 0:64, ctx rows 64:128 (V_aug = [1|V])
            nc.sync.dma_start(out=sums[0:64, :], in_=he[64:128, qt, :])
            nc.sync.dma_start(out=sums[64:128, :], in_=ho[0:64, qt, :])
            nc.vector.reciprocal_approx_fast(sums[:], sums[:])   # in place
            nc.vector.tensor_mul(ctxn[0:64, c, qt, :], he[0:64, qt, :],
                                 sums[0:64, :])
            nc.vector.tensor_mul(ctxn[64:128, c, qt, :], ho[64:128, qt, :],
                                 sums[64:128, :])

        def proj_group(t16, no):
            def emit():
                tag = f"y{t16}"
                if no == 0:
                    y_tiles[t16] = yp.tile([128, D], F32, tag="y", name=f"y_{t16}")
                ps = ps_pool.tile([128, 512], F32, tag="p1", name=f"yps_{t16}_{no}")
                qt, o = t16 // 4, 128 * (t16 % 4)
                for c4 in range(4):
                    nc.tensor.matmul(ps[:], lhsT=ctxn[:, c4, qt, o:o + 128],
                                     rhs=wo_sb[:, c4, 512 * no:512 * (no + 1)],
                                     start=(c4 == 0), stop=(c4 == 3))
                nc.vector.tensor_add(y_tiles[t16][:, 512 * no:512 * (no + 1)], ps[:],
                                     bo_bc[:, 512 * no:512 * (no + 1)])
                if no == 1:
                    nc.sync.dma_start(out=y_d[128 * t16:128 * (t16 + 1), :],
                                      in_=y_tiles[t16][:])
            return emit

        y_tiles = {}

        # ---- interleaved schedule ----
        for c in range(4):
            for qt in range(4):
                if c == 0:
                    qk_tile(0, qt)
                    qk_tile(4, qt)
                if c < 2:
                    fillers = [lambda m=c + 1, n=qt: qk_tile(m, n),
                               lambda m=5 + c, n=qt: qk_tile(m, n)]
                elif c == 2:
                    fillers = [lambda m=(3 if qt % 2 == 0 else 7), n=qt // 2:
                               qk_tile(m, n)]
                else:
                    fillers = []
                    if qt < 2:
                        fillers += [lambda n=qt + 2: qk_tile(3, n),
                                    lambda n=qt + 2: qk_tile(7, n)]
                    if qt > 0:
                        fillers += [proj_group(t16, no)
                                    for t16 in range(4 * (qt - 1), 4 * qt)
                                    for no in range(2)]
                attention_qt(c, qt, fillers)
                normalize(c, qt)
        proj3 = [proj_group(t16, no) for t16 in range(12, 16) for no in range(2)]
        for f in proj3:
            f()

    nc.compile()
    return nc


def _reference_np(x, W_qkv, b_qkv, W_o, b_o, key_padding_mask):
    """Numpy fallback for inputs that do not match the compiled assumptions."""
    b_, t_, d_ = x.shape
    hd = d_ // H
    qkv = x.astype(np.float64) @ W_qkv.astype(np.float64) + b_qkv
    q, k, v = np.split(qkv, 3, axis=-1)

    def heads(t):
        return t.reshape(b_, t_, H, hd).transpose(0, 2, 1, 3)

    q, k, v = heads(q), heads(k), heads(v)
    s = np.einsum("bhqd,bhkd->bhqk", q, k) / np.sqrt(hd)
    causal = np.triu(np.ones((t_, t_), bool), k=1)
    mask = key_padding_mask[:, None, None, :] | causal[None, None]
    s = np.where(mask, -np.inf, s)
    s = s - s.max(axis=-1, keepdims=True)
    e = np.exp(s)
    with np.errstate(invalid="ignore"):
        a = e / e.sum(axis=-1, keepdims=True)
    ctx = np.einsum("bhqk,bhkd->bhqd", a, v)
    y = ctx.transpose(0, 2, 1, 3).reshape(b_, t_, d_) @ W_o.astype(np.float64) + b_o
    return y.astype(np.float32)


def kernel(x, W_qkv, b_qkv, W_o, b_o, key_padding_mask):
    x = np.asarray(x)
    W_qkv, b_qkv = np.asarray(W_qkv), np.asarray(b_qkv)
    W_o, b_o = np.asarray(W_o), np.asarray(b_o)
    key_padding_mask = np.asarray(key_padding_mask)

    expected_mask = np.zeros((B, T), bool)
    expected_mask[:, T - NPAD:] = True
    if (x.shape != (B, T, D) or not np.array_equal(key_padding_mask, expected_mask)):
        return _reference_np(x, W_qkv, b_qkv, W_o, b_o, key_padding_mask)

    if "nc" not in _CACHE:
        _CACHE["nc"] = _build()
    nc = _CACHE["nc"]

    bf = ml_dtypes.bfloat16
    in_maps = []
    for c in range(N_CORES):
        b, g = divmod(c, 2)
        cols = slice(g * GD, (g + 1) * GD)
        wq = np.concatenate([W_qkv[:, cols], W_qkv[:, D + g * GD:D + (g + 1) * GD],
                             W_qkv[:, 2 * D + g * GD:2 * D + (g + 1) * GD]],
                            axis=1).astype(bf)
        bq = np.concatenate([b_qkv[cols], b_qkv[D + g * GD:D + (g + 1) * GD]])
        xT = np.ascontiguousarray(x[b].T).astype(bf)
        # pack wq columns: m0 | m4 | V | m1 m5 m2 m6 m3 m7 (d-major inside)
        wq_blocks = []
        for m in (0, 4):
            wq_blocks += [wq[128 * d:128 * (d + 1), 128 * m:128 * (m + 1)]
                          for d in range(8)]
        wq_blocks += [wq[128 * d:128 * (d + 1), 1024:1536] for d in range(8)]
        for m in (1, 5, 2, 6, 3, 7):
            wq_blocks += [wq[128 * d:128 * (d + 1), 128 * m:128 * (m + 1)]
                          for d in range(8)]
        wq_p = np.concatenate(wq_blocks, axis=1)
        # pack xT columns: (nt, d) blocks of 512 tokens
        xT_p = np.concatenate([xT[128 * d:128 * (d + 1), 512 * nt:512 * (nt + 1)]
                               for nt in range(4) for d in range(8)], axis=1)
        in_maps.append({
            "xT": np.ascontiguousarray(xT_p),
            "wqkv": np.ascontiguousarray(wq_p),
            "wo": np.ascontiguousarray(W_o[g * GD:(g + 1) * GD, :]).astype(bf),
            "bqk": np.ascontiguousarray(bq.reshape(8, 128).T.astype(np.float32)),
            "bv": np.ascontiguousarray(b_qkv[2 * D + g * GD:2 * D + (g + 1) * GD]).astype(np.float32),
            "bo": np.ascontiguousarray(b_o).astype(np.float32),
        })

    trace = bool(os.environ.get("MHA_TRACE"))
    if trace:
        _register_ntff_hook()
    res = run_bass_kernel_spmd(nc, in_maps, core_ids=list(range(N_CORES)),
                               trace=trace)
    if trace:
        _CACHE["exec_time_ns"] = res.exec_time_ns

    y = np.empty((B, T, D), np.float32)
    for b in range(B):
        y[b] = res.results[2 * b]["y"] + res.results[2 * b + 1]["y"]
    return y


def _register_ntff_hook():
    """antenv.axon_hooks is absent in this container; synthesize it so
    run_bass_kernel_spmd(trace=True) can NTFF-profile via ctypes."""
    import types

    if "antenv.axon_hooks" in sys.modules:
        return
    sys.path.insert(0, "/root/.axon_site")
    from trn_agent_boot.trn_boot import _ntff_profile_via_ctypes

    hook = _ntff_profile_via_ctypes("/opt/axon/libaxon_pjrt.so")
    mod = types.ModuleType("antenv.axon_hooks")
    mod._hook = hook
    mod.get_axon_ntff_profile_hook = lambda: mod._hook
    mod.set_axon_ntff_profile_hook = lambda h: setattr(mod, "_hook", h)
    sys.modules["antenv.axon_hooks"] = mod


# revision 20
# speedup vs baseline: 1.0207x; 1.0207x over previous
"""Masked multi-head attention (B=4, T=2048, D=1024, H=16) on 8 trn2 NeuronCores.

Sharding: core c handles batch b = c//2 and head-group g = c%2 (8 heads, 512
of the 1024 model dims).  Each core runs the fused QKV projection for its
head-group over its batch, causal+padding-masked attention for its 8 heads,
and a partial out-projection (its 512 rows of W_o).  The two cores of a batch
produce additive partials of y[b]; the host sums the pair (0.6% of FLOPs).

Device algorithm (per core), all matmuls bf16 with f32 PSUM accumulation:
  - qT,kT  = (x @ Wq|k)^T computed directly in [dims, tok] layout
             (lhsT = W chunk, rhs = xT chunk), bias added per-partition.
  - V      computed in natural [tok, dims] layout (lhsT = xT chunk,
             rhs = Wv), packed into V_aug = [V | 1] (even heads) or [1 | V]
             (odd heads) so A@V_aug also yields the softmax row-sums
             replicated across 64 partitions.
  - scores S^T[k, q] per 128-key block kb: lhsT = kT block, rhs = qT.
             Keys >= 1792 are fully padded -> those blocks never computed.
             Causal: only q >= 128*kb computed; exp(S/8) via ScalarE into
             bf16; diagonal band masked multiplicatively.
  - ctx^T  accumulated over key blocks in PSUM; row-sums come free via the
             V_aug ones-columns; reciprocal on VectorE; normalize into bf16.
  - y      = ctx @ W_o rows (natural layout) + b_o broadcast, f32 out.

Scheduling: everything is emitted as one interleaved stream.  Attention is
processed q-tile-major per head pair with scores -> exp -> A@V interleaved at
key-block granularity; QKV projection tiles, V tiles and the out-projection
are woven between key blocks as PE filler so the tensor engine never idles
long enough for the HAM clock gate to drop it to 1.2 GHz.  Host-side, wq/xT
are packed into the exact SBUF block layout so every DMA wave is one large
contiguous transfer ordered by first use.
"""

import os
import sys

sys.path.insert(0, "/opt/trn_rl_repo")

from contextlib import ExitStack

import ml_dtypes
import numpy as np

import concourse.bass as bass
import concourse.tile as tile
from concourse import bacc, mybir
from concourse.bass_utils import run_bass_kernel_spmd

B, T, D, H, HD = 4, 2048, 1024, 16, 64
N_CORES = 8
NH = H // 2            # heads per core = 8
GD = NH * HD           # head-group width = 512
TK = 14                # valid 128-key blocks (keys < 1792; rest padded)
NPAD = 256             # padded key positions at the end
BF16 = mybir.dt.bfloat16
F32 = mybir.dt.float32
AF = mybir.ActivationFunctionType

_CACHE = {}


def _build():
    nc = bacc.Bacc("TRN2", target_bir_lowering=False, debug=False,
                   num_devices=N_CORES)
    # xT packed as [128, (nt, d) blocks of 512]; wq packed as
    # [128, m0|m4|V|m1|m5|m2|m6|m3|m7 blocks] -- both host-reordered so every
    # DMA wave is fully contiguous (large descriptors, ordered by first use).
    xT_d = nc.dram_tensor("xT", [128, 8 * T], BF16, kind="ExternalInput").ap()
    wqkv_d = nc.dram_tensor("wqkv", [128, 8 * 3 * GD // 128 * 128], BF16,
                            kind="ExternalInput").ap()
    wo_d = nc.dram_tensor("wo", [GD, D], BF16, kind="ExternalInput").ap()
    bqk_d = nc.dram_tensor("bqk", [128, 8], F32, kind="ExternalInput").ap()
    bv_d = nc.dram_tensor("bv", [GD], F32, kind="ExternalInput").ap()
    bo_d = nc.dram_tensor("bo", [D], F32, kind="ExternalInput").ap()
    y_d = nc.dram_tensor("y", [T, D], F32, kind="ExternalOutput").ap()

    def bcast128(src_ap):
        """DMA access pattern replicating a 1-D dram vector over 128 partitions."""
        return bass.AP(tensor=src_ap.tensor, offset=src_ap.offset,
                       ap=[[0, 128]] + list(src_ap.ap))

    with tile.TileContext(nc) as tc, ExitStack() as ctx:
        pers = ctx.enter_context(tc.tile_pool(name="pers", bufs=1))
        ps_pool = ctx.enter_context(tc.tile_pool(name="ps", bufs=2, space="PSUM"))
        esp = ctx.enter_context(tc.tile_pool(name="es", bufs=6))
        stgp = ctx.enter_context(tc.tile_pool(name="stg", bufs=2))
        nrmp = ctx.enter_context(tc.tile_pool(name="nrm", bufs=1))
        yp = ctx.enter_context(tc.tile_pool(name="yp", bufs=2))

        # ---- persistent tiles ----
        wo_sb = pers.tile([128, 4, D], BF16)          # W_o rows, 4 chunks of 128
        bqk_sb = pers.tile([128, 8], F32)             # q|k bias per col-tile
        bv_bc = pers.tile([128, GD], F32)             # v bias bcast over tokens
        bo_bc = pers.tile([128, D], F32)              # out bias bcast over tokens
        band = pers.tile([128, 1024], BF16)           # band[k, i] = 1 iff i-512 >= k
        qk_sb = pers.tile([128, 8, T], BF16)          # m<4: qT pairs, m>=4: kT
        vaug = pers.tile([128, 2, 4, TK, 128], BF16)  # V_aug[par, hp, key chunk]
        xT_sb = pers.tile([128, 8 * T], BF16)         # packed (nt, d) blocks
        wq_sb = pers.tile([128, 12 * 1024], BF16)     # packed m/V blocks

        QKOFF = {0: 0, 4: 1024, 1: 6144, 5: 7168, 2: 8192, 6: 9216,
                 3: 10240, 7: 11264}
        VOFF = 2048

        def wq_qk(m, d8):
            return wq_sb[:, QKOFF[m] + 128 * d8:QKOFF[m] + 128 * (d8 + 1)]

        def xT_nt(nt, d8):
            return xT_sb[:, (nt * 8 + d8) * 512:(nt * 8 + d8) * 512 + 512]
        ctxn = pers.tile([128, 4, 4, 512], BF16)      # normalized ctx^T chunks

        # ---- loads: contiguous waves ordered by first use, split in half so
        #      two DMA queues work each wave ----
        def wave(sb, dram, lo, hi):
            mid = (lo + hi) // 2
            nc.sync.dma_start(out=sb[:, lo:mid], in_=dram[:, lo:mid])
            nc.sync.dma_start(out=sb[:, mid:hi], in_=dram[:, mid:hi])

        wave(wq_sb, wqkv_d, 0, 2048)            # m0 + m4
        wave(xT_sb, xT_d, 0, 2048)              # nt0 d0..3
        wave(xT_sb, xT_d, 2048, 4096)           # nt0 d4..7
        nc.sync.dma_start(out=bqk_sb[:], in_=bqk_d)
        nc.sync.dma_start(out=bv_bc[:], in_=bcast128(bv_d))
        wave(wq_sb, wqkv_d, 2048, 6144)         # V columns
        wave(xT_sb, xT_d, 4096, 8192)           # nt1
        wave(wq_sb, wqkv_d, 6144, 8192)         # m1 + m5
        wave(xT_sb, xT_d, 8192, 12288)          # nt2
        wave(wq_sb, wqkv_d, 8192, 10240)        # m2 + m6
        wave(xT_sb, xT_d, 12288, 16384)         # nt3
        wave(wq_sb, wqkv_d, 10240, 12288)       # m3 + m7
        for c4 in range(4):
            nc.sync.dma_start(out=wo_sb[:, c4, :], in_=wo_d[128 * c4:128 * (c4 + 1), :])
        nc.sync.dma_start(out=bo_bc[:], in_=bcast128(bo_d))
        nc.vector.memset(band[:], 1.0)
        # keep 1.0 where (i - 512) - k >= 0 else 0.0
        nc.gpsimd.affine_select(out=band[:], in_=band[:],
                                compare_op=mybir.AluOpType.is_ge, fill=0.0,
                                base=-512, pattern=[[1, 1024]], channel_multiplier=-1)
        nc.vector.memset(vaug[:, 0, :, :, 64:128], 1.0)   # even heads: [V | 1]
        nc.vector.memset(vaug[:, 1, :, :, 0:64], 1.0)     # odd heads:  [1 | V]

        # ---- QKV projection pieces, emitted as PE fillers ----
        def qk_tile(m, nt):
            # k columns (m >= 4) beyond token 1792 are fully padded: never read
            w = 256 if (m >= 4 and nt == 3) else 512
            ps = ps_pool.tile([128, 512], F32, tag="p1", name=f"p1_{m}_{nt}")
            for d8 in range(8):
                nc.tensor.matmul(ps[:, 0:w], lhsT=wq_qk(m, d8),
                                 rhs=xT_nt(nt, d8)[:, 0:w],
                                 start=(d8 == 0), stop=(d8 == 7))
            nc.vector.tensor_scalar_add(qk_sb[:, m, 512 * nt:512 * nt + w],
                                        ps[:, 0:w], bqk_sb[:, m:m + 1])

        def v_tile(t16):
            ps = ps_pool.tile([128, 512], F32, tag="p1", name=f"p1v_{t16}")
            nt, to = t16 // 4, 128 * (t16 % 4)
            for d8 in range(8):
                nc.tensor.matmul(ps[:],
                                 lhsT=xT_sb[:, (nt * 8 + d8) * 512 + to:(nt * 8 + d8) * 512 + to + 128],
                                 rhs=wq_sb[:, VOFF + 512 * d8:VOFF + 512 * (d8 + 1)],
                                 start=(d8 == 0), stop=(d8 == 7))
            psv = ps.rearrange("p (hp par d) -> p hp par d", par=2, d=64)
            bvv = bv_bc.rearrange("p (hp par d) -> p hp par d", par=2, d=64)
            nc.vector.tensor_add(vaug[:, 0, :, t16, 0:64], psv[:, :, 0, :],
                                 bvv[:, :, 0, :])
            nc.vector.tensor_add(vaug[:, 1, :, t16, 64:128], psv[:, :, 1, :],
                                 bvv[:, :, 1, :])

        stg_tiles = {}  # h -> stage tile

        def attention_qt(c, qt, fillers=()):
            """Scores + exp + A@V_aug for q-tile qt of head pair c, interleaved
            per key block so ScalarE exp overlaps the PE matmuls.  The two
            heads occupy PE row-groups 0/64 (concurrent matmuls) and the two
            halves of shared score/exp tiles.  `fillers` are independent PE
            work (QKV tiles / out-projection) woven between key blocks to
            absorb the exp latency."""
            kmax = min(4 * qt + 3, TK - 1)
            fillers = list(fillers)
            fill_every = max(1, (kmax + 1) // (len(fillers) + 1)) if fillers else 0
            cps = [ps_pool.tile([128, 512], F32, tag="cps", name=f"cps_{c}_{qt}_{i}")
                   for i in range(2)]
            for kb in range(kmax + 1):
                if c == 0 and qt == kb // 4:   # JIT V chunks during pair 0
                    v_tile(kb)
                # diagonal blocks only need columns q >= 128*kb of the q-tile;
                # the two parities' valid columns are packed next to each other
                # so exp touches only live elements
                off = max(0, 128 * kb - 512 * qt)
                w = 512 - off
                psc = ps_pool.tile([128, 1024], F32, tag="sc", name=f"sc_{c}_{qt}_{kb}")
                for par in (0, 1):
                    r = 64 * par
                    nc.tensor.matmul(
                        psc[:, 512 * par:512 * par + w],
                        lhsT=qk_sb[r:r + 64, 4 + c, 128 * kb:128 * (kb + 1)],
                        rhs=qk_sb[r:r + 64, c, 512 * qt + off:512 * (qt + 1)],
                        start=True, stop=True)
                est = esp.tile([128, 1024], BF16, tag="es", name=f"es_{c}_{qt}_{kb}")
                nc.scalar.activation(est[:, 0:512 + w], psc[:, 0:512 + w], AF.Exp,
                                     scale=float(1.0 / np.sqrt(HD)))
                if kb >= 4 * qt:  # mask the causal triangle of the diagonal block
                    for par in (0, 1):
                        nc.vector.tensor_mul(est[:, 512 * par:512 * par + w],
                                             est[:, 512 * par:512 * par + w],
                                             band[:, 512:512 + w])
                for par in (0, 1):
                    nc.tensor.matmul(cps[par][:, off:512],
                                     lhsT=vaug[:, par, c, kb, :],
                                     rhs=est[:, 512 * par:512 * par + w],
                                     start=(kb == 0), stop=(kb == kmax))
                if fillers and fill_every and kb % fill_every == fill_every - 1:
                    fillers.pop(0)()
            for f in fillers:
                f()
            for par in (0, 1):
                h = 2 * c + par
                if qt == 0:
                    stg_tiles[h] = stgp.tile([128, 4, 512], F32, tag="stg",
                                             name=f"stg_{h}")
                nc.vector.tensor_copy(stg_tiles[h][:, qt, :], cps[par][:])

        def normalize(c, qt):
            he, ho = stg_tiles[2 * c], stg_tiles[2 * c + 1]
            sums = nrmp.tile([128, 512], F32, tag="sums", name=f"sums_{c}_{qt}",
                             bufs=2)
            # even head: ctx rows 0:64, sums rows 64:128 (V_aug = [V|1])
            # odd head:  sums rows 0:64, ctx rows 64:128 (V_aug = [1|V])
            nc.sync.dma_start(out=sums[0:64, :], in_=he[64:128, qt, :])
            nc.sync.dma_start(out=sums[64:128, :], in_=ho[0:64, qt, :])
            nc.vector.reciprocal_approx_fast(sums[:], sums[:])   # in place
            nc.vector.tensor_mul(ctxn[0:64, c, qt, :], he[0:64, qt, :],
                                 sums[0:64, :])
            nc.vector.tensor_mul(ctxn[64:128, c, qt, :], ho[64:128, qt, :],
                                 sums[64:128, :])

        def proj_group(t16, no):
            def emit():
                tag = f"y{t16}"
                if no == 0:
                    y_tiles[t16] = yp.tile([128, D], F32, tag="y", name=f"y_{t16}")
                ps = ps_pool.tile([128, 512], F32, tag="p1", name=f"yps_{t16}_{no}")
                qt, o = t16 // 4, 128 * (t16 % 4)
                for c4 in range(4):
                    nc.tensor.matmul(ps[:], lhsT=ctxn[:, c4, qt, o:o + 128],
                                     rhs=wo_sb[:, c4, 512 * no:512 * (no + 1)],
                                     start=(c4 == 0), stop=(c4 == 3))
                nc.vector.tensor_add(y_tiles[t16][:, 512 * no:512 * (no + 1)], ps[:],
                                     bo_bc[:, 512 * no:512 * (no + 1)])
                if no == 1:
                    nc.sync.dma_start(out=y_d[128 * t16:128 * (t16 + 1), :],
                                      in_=y_tiles[t16][:])
            return emit

        y_tiles = {}

        # ---- interleaved schedule ----
        for c in range(4):
            for qt in range(4):
                if c == 0:
                    qk_tile(0, qt)
                    qk_tile(4, qt)
                if c < 2:
                    fillers = [lambda m=c + 1, n=qt: qk_tile(m, n),
                               lambda m=5 + c, n=qt: qk_tile(m, n)]
                elif c == 2:
                    fillers = [lambda m=(3 if qt % 2 == 0 else 7), n=qt // 2:
                               qk_tile(m, n)]
                else:
                    fillers = []
                    if qt < 2:
                        fillers += [lambda n=qt + 2: qk_tile(3, n),
                                    lambda n=qt + 2: qk_tile(7, n)]
                    if qt > 0:
                        fillers += [proj_group(t16, no)
                                    for t16 in range(4 * (qt - 1), 4 * qt)
                                    for no in range(2)]
                attention_qt(c, qt, fillers)
                normalize(c, qt)
        proj3 = [proj_group(t16, no) for t16 in range(12, 16) for no in range(2)]
        for f in proj3:
            f()

    nc.compile()
    return nc


def _reference_np(x, W_qkv, b_qkv, W_o, b_o, key_padding_mask):
    """Numpy fallback for inputs that do not match the compiled assumptions."""
    b_, t_, d_ = x.shape
    hd = d_ // H
    qkv = x.astype(np.float64) @ W_qkv.astype(np.float64) + b_qkv
    q, k, v = np.split(qkv, 3, axis=-1)

    def heads(t):
        return t.reshape(b_, t_, H, hd).transpose(0, 2, 1, 3)

    q, k, v = heads(q), heads(k), heads(v)
    s = np.einsum("bhqd,bhkd->bhqk", q, k) / np.sqrt(hd)
    causal = np.triu(np.ones((t_, t_), bool), k=1)
    mask = key_padding_mask[:, None, None, :] | causal[None, None]
    s = np.where(mask, -np.inf, s)
    s = s - s.max(axis=-1, keepdims=True)
    e = np.exp(s)
    with np.errstate(invalid="ignore"):
        a = e / e.sum(axis=-1, keepdims=True)
    ctx = np.einsum("bhqk,bhkd->bhqd", a, v)
    y = ctx.transpose(0, 2, 1, 3).reshape(b_, t_, d_) @ W_o.astype(np.float64) + b_o
    return y.astype(np.float32)


def kernel(x, W_qkv, b_qkv, W_o, b_o, key_padding_mask):
    x = np.asarray(x)
    W_qkv, b_qkv = np.asarray(W_qkv), np.asarray(b_qkv)
    W_o, b_o = np.asarray(W_o), np.asarray(b_o)
    key_padding_mask = np.asarray(key_padding_mask)

    expected_mask = np.zeros((B, T), bool)
    expected_mask[:, T - NPAD:] = True
    if (x.shape != (B, T, D) or not np.array_equal(key_padding_mask, expected_mask)):
        return _reference_np(x, W_qkv, b_qkv, W_o, b_o, key_padding_mask)

    if "nc" not in _CACHE:
        _CACHE["nc"] = _build()
    nc = _CACHE["nc"]

    bf = ml_dtypes.bfloat16
    in_maps = []
    for c in range(N_CORES):
        b, g = divmod(c, 2)
        cols = slice(g * GD, (g + 1) * GD)
        wq = np.concatenate([W_qkv[:, cols], W_qkv[:, D + g * GD:D + (g + 1) * GD],
                             W_qkv[:, 2 * D + g * GD:2 * D + (g + 1) * GD]],
                            axis=1).astype(bf)
        bq = np.concatenate([b_qkv[cols], b_qkv[D + g * GD:D + (g + 1) * GD]])
        xT = np.ascontiguousarray(x[b].T).astype(bf)
        # pack wq columns: m0 | m4 | V | m1 m5 m2 m6 m3 m7 (d-major inside)
        wq_blocks = []
        for m in (0, 4):
            wq_blocks += [wq[128 * d:128 * (d + 1), 128 * m:128 * (m + 1)]
                          for d in range(8)]
        wq_blocks += [wq[128 * d:128 * (d + 1), 1024:1536] for d in range(8)]
        for m in (1, 5, 2, 6, 3, 7):
            wq_blocks += [wq[128 * d:128 * (d + 1), 128 * m:128 * (m + 1)]
                          for d in range(8)]
        wq_p = np.concatenate(wq_blocks, axis=1)
        # pack xT columns: (nt, d) blocks of 512 tokens
        xT_p = np.concatenate([xT[128 * d:128 * (d + 1), 512 * nt:512 * (nt + 1)]
                               for nt in range(4) for d in range(8)], axis=1)
        in_maps.append({
            "xT": np.ascontiguousarray(xT_p),
            "wqkv": np.ascontiguousarray(wq_p),
            "wo": np.ascontiguousarray(W_o[g * GD:(g + 1) * GD, :]).astype(bf),
            "bqk": np.ascontiguousarray(bq.reshape(8, 128).T.astype(np.float32)),
            "bv": np.ascontiguousarray(b_qkv[2 * D + g * GD:2 * D + (g + 1) * GD]).astype(np.float32),
            "bo": np.ascontiguousarray(b_o).astype(np.float32),
        })

    trace = bool(os.environ.get("MHA_TRACE"))
    if trace:
        _register_ntff_hook()
    res = run_bass_kernel_spmd(nc, in_maps, core_ids=list(range(N_CORES)),
                               trace=trace)
    if trace:
        _CACHE["exec_time_ns"] = res.exec_time_ns

    y = np.empty((B, T, D), np.float32)
    for b in range(B):
        y[b] = res.results[2 * b]["y"] + res.results[2 * b + 1]["y"]
    return y


def _register_ntff_hook():
    """antenv.axon_hooks is absent in this container; synthesize it so
    run_bass_kernel_spmd(trace=True) can NTFF-profile via ctypes."""
    import types

    if "antenv.axon_hooks" in sys.modules:
        return
    sys.path.insert(0, "/root/.axon_site")
    from trn_agent_boot.trn_boot import _ntff_profile_via_ctypes

    hook = _ntff_profile_via_ctypes("/opt/axon/libaxon_pjrt.so")
    mod = types.ModuleType("antenv.axon_hooks")
    mod._hook = hook
    mod.get_axon_ntff_profile_hook = lambda: mod._hook
    mod.set_axon_ntff_profile_hook = lambda h: setattr(mod, "_hook", h)
    sys.modules["antenv.axon_hooks"] = mod
